# revision 3
# baseline (speedup 1.0000x reference)
"""Causal self-attention (GQA + RoPE) Trainium2 kernel, 8-way sharded.

Sharding: DP=4 over batch x TP=2 over kv-head groups (2 kv heads + their
8 q heads per group).  Each core computes its batch's qkv projection for
its head group, causal flash-style attention, and a partial c_proj
(columns of w_proj for its head group).  Host sums the two partial
c_proj outputs per batch.

Everything on-chip runs transposed ([feature, token] layout) so matmuls
contract along partitions; host transposes inputs/outputs.

RoPE trick: w_attn q/k rows are permuted per-head to [even dims; odd
dims] so the rotation pairs (2f, 2f+1) land in partition f and f+64 of
the qkv psum tile; the rotation is then 6 DVE ops per tile (PSUM inputs
may sit at a different base partition than SBUF inputs, which makes
this legal).  The q.k dot product is invariant to the shared
permutation; v is unpermuted.

Softmax: att^T tiles ([k, q] layout) are exp'd on ACT without
max-subtraction (logits are O(6), fp32-safe); denominators accumulate
via ones-column matmuls; the per-q reciprocal is broadcast down
partitions with a f32r outer-product matmul.
"""

import math

import numpy as np
import ml_dtypes

import concourse.bass as bass
import concourse.mybir as mybir
import concourse.tile as tile
from concourse import bacc
from concourse.bass_utils import run_bass_kernel_spmd

ALU = mybir.AluOpType
AF = mybir.ActivationFunctionType
F32 = mybir.dt.float32
F32R = mybir.dt.float32r
BF16 = mybir.dt.bfloat16
BF = ml_dtypes.bfloat16

# problem shape (hardcoded per contest rules)
B, T, C = 4, 2048, 2048
N_HEAD, N_KV_HEAD, HD = 16, 4, 128
ROPE_THETA = 10000.0

TP = 2            # head-group shards
DP = 4            # batch shards
HQ = N_HEAD // TP         # 8 q heads per core
HKV = N_KV_HEAD // TP     # 2 kv heads per core
NREP = N_HEAD // N_KV_HEAD  # 4
QK_ROWS = (HQ + HKV) * HD   # 1280
KC = C // 128     # 16 contraction tiles
NQ = T // 512     # 4 token strips
MQK = QK_ROWS // 128  # 10 feature tiles (8 q heads + 2 kv heads)
SCALE = 1.0 / math.sqrt(HD)

N_CORES = 8

_NC = None        # cached compiled Bass module
LAST_RUN = None   # BassKernelResults of the most recent kernel() call


def build_nc():
    nc = bacc.Bacc(None, target_bir_lowering=False, debug=False)

    xT = nc.declare_dram_parameter("xT", [C, T], BF16, isOutput=False)
    wqkT = nc.declare_dram_parameter("wqkT", [C, QK_ROWS], BF16, isOutput=False)
    wvT = nc.declare_dram_parameter("wvT", [C, HKV * HD], BF16, isOutput=False)
    wpT = nc.declare_dram_parameter("wpT", [HQ * HD, C], BF16, isOutput=False)
    trigc = nc.declare_dram_parameter("trigc", [64, T], F32, isOutput=False)
    trigs = nc.declare_dram_parameter("trigs", [64, T], F32, isOutput=False)
    maskd = nc.declare_dram_parameter("maskd", [4, 128, 512], BF16, isOutput=False)
    outT = nc.declare_dram_parameter("outT", [C, T], F32, isOutput=True)

    with tile.TileContext(nc) as tc:
        with (
            tc.tile_pool(name="const", bufs=1) as const,
            tc.tile_pool(name="persist", bufs=1) as persist,
        ):
            trigc_sb = const.tile([64, T], F32, name="trigc")
            trigs_sb = const.tile([64, T], F32, name="trigs")
            mask_sb = const.tile([128, 4, 512], BF16, name="mask")
            ones_col = const.tile([128, 1], BF16, name="onec")
            ones_row_f = const.tile([1, 128], F32, name="onerf")
            ones_row = const.tile([1, 128], F32R, name="oner")

            nc.sync.dma_start(trigc_sb[:], trigc[:])
            nc.sync.dma_start(trigs_sb[:], trigs[:])
            nc.sync.dma_start(mask_sb[:], maskd.rearrange("d p q -> p d q"))
            nc.vector.memset(ones_col[:], 1.0)
            nc.vector.memset(ones_row_f[:], 1.0)
            with nc.allow_low_precision("f32r ones for recip broadcast"):
                nc.vector.tensor_copy(ones_row[:], ones_row_f[:])

            qrot = [persist.tile([128, T], BF16, name=f"qrot{h}") for h in range(HQ)]
            krot = [persist.tile([128, T], BF16, name=f"krot{h}") for h in range(HKV)]
            v_sb = persist.tile([128, T // 128, HKV * HD], BF16, name="vtok")
            yt = persist.tile([128, HQ, T], BF16, name="yt")

            # ---------------- Phase A: qkv projection + RoPE ----------------
            with (
                tc.tile_pool(name="wa", bufs=1) as wa,
                tc.tile_pool(name="xa", bufs=2) as xa,
                tc.tile_pool(name="ta", bufs=4) as ta,
                tc.tile_pool(name="psA", bufs=2, space="PSUM") as psA,
                tc.tile_pool(name="psV", bufs=2, space="PSUM") as psV,
            ):
                wqk_sb = wa.tile([128, KC, QK_ROWS], BF16, name="wqk")
                wv_sb = wa.tile([128, KC, HKV * HD], BF16, name="wv")
                for kc in range(KC):
                    nc.sync.dma_start(
                        wqk_sb[:, kc, :], wqkT[kc * 128 : (kc + 1) * 128, :]
                    )
                    nc.sync.dma_start(
                        wv_sb[:, kc, :], wvT[kc * 128 : (kc + 1) * 128, :]
                    )

                for n in range(NQ):
                    nsl = bass.ts(n, 512)
                    xs = xa.tile([128, KC, 512], BF16, name="xs")
                    for kc in range(KC):
                        nc.sync.dma_start(
                            xs[:, kc, :], xT[kc * 128 : (kc + 1) * 128, nsl]
                        )
                    for m in range(MQK):
                        ps = psA.tile([128, 512], F32, name="psA")
                        for kc in range(KC):
                            nc.tensor.matmul(
                                ps[:],
                                wqk_sb[:, kc, m * 128 : (m + 1) * 128],
                                xs[:, kc, :],
                                start=(kc == 0),
                                stop=(kc == KC - 1),
                            )
                        dst = qrot[m] if m < HQ else krot[m - HQ]
                        cs = trigc_sb[:, nsl]
                        ss = trigs_sb[:, nsl]
                        t1 = ta.tile([64, 512], F32, name="t")
                        t2 = ta.tile([64, 512], F32, name="t")
                        t3 = ta.tile([64, 512], F32, name="t")
                        t4 = ta.tile([64, 512], F32, name="t")
                        nc.vector.tensor_tensor(t1[:], ps[0:64, :], cs, ALU.mult)
                        nc.vector.tensor_tensor(t2[:], ps[64:128, :], ss, ALU.mult)
                        nc.vector.tensor_tensor(
                            dst[0:64, nsl], t1[:], t2[:], ALU.subtract
                        )
                        nc.vector.tensor_tensor(t3[:], ps[0:64, :], ss, ALU.mult)
                        nc.vector.tensor_tensor(t4[:], ps[64:128, :], cs, ALU.mult)
                        nc.vector.tensor_tensor(
                            dst[64:128, nsl], t3[:], t4[:], ALU.add
                        )
                    # v for this strip, token-major
                    for j in range(4):
                        tt = n * 4 + j
                        psv = psV.tile([128, HKV * HD], F32, name="psv")
                        for kc in range(KC):
                            nc.tensor.matmul(
                                psv[:],
                                xs[:, kc, j * 128 : (j + 1) * 128],
                                wv_sb[:, kc, :],
                                start=(kc == 0),
                                stop=(kc == KC - 1),
                            )
                        nc.scalar.copy(v_sb[:, tt, :], psv[:])

            # ---------------- Phase B: causal attention ----------------
            with tc.tile_pool(name="wp", bufs=1) as wp:
                # prefetch w_proj during phase B
                wp_sb = wp.tile([128, HQ, C], BF16, name="wp")
                for h in range(HQ):
                    nc.sync.dma_start(
                        wp_sb[:, h, :], wpT[h * 128 : (h + 1) * 128, :]
                    )

                with (
                    tc.tile_pool(name="eb", bufs=3) as eb,
                    tc.tile_pool(name="rb", bufs=2) as rb,
                    tc.tile_pool(name="psS", bufs=2, space="PSUM") as psS,
                    tc.tile_pool(name="psY", bufs=2, space="PSUM") as psY,
                    tc.tile_pool(name="psD", bufs=2, space="PSUM") as psD,
                    tc.tile_pool(name="psR", bufs=1, space="PSUM") as psR,
                ):
                    for h in range(HQ):
                        kvh = h // NREP
                        for qj in range(NQ):
                            qsl = bass.ts(qj, 512)
                            ps_y = psY.tile([128, 512], F32, name="psy")
                            ps_d = psD.tile([1, 512], F32, name="psd")
                            nkt = 4 * qj + 4
                            for kt in range(nkt):
                                ps_s = psS.tile([128, 512], F32, name="pss")
                                nc.tensor.matmul(
                                    ps_s[:],
                                    krot[kvh][:, kt * 128 : (kt + 1) * 128],
                                    qrot[h][:, qsl],
                                    start=True,
                                    stop=True,
                                )
                                e = eb.tile([128, 512], BF16, name="e")
                                nc.scalar.activation(
                                    e[:], ps_s[:], AF.Exp, scale=SCALE
                                )
                                d = kt - 4 * qj
                                if d >= 0:
                                    nc.vector.tensor_tensor(
                                        e[:], e[:], mask_sb[:, d, :], ALU.mult
                                    )
                                nc.tensor.matmul(
                                    ps_y[:],
                                    v_sb[:, kt, kvh * HD : (kvh + 1) * HD],
                                    e[:],
                                    start=(kt == 0),
                                    stop=(kt == nkt - 1),
                                )
                                nc.tensor.matmul(
                                    ps_d[:],
                                    ones_col[:],
                                    e[:],
                                    start=(kt == 0),
                                    stop=(kt == nkt - 1),
                                )
                            rec_f = rb.tile([1, 512], F32, name="recf")
                            rec_r = rb.tile([1, 512], F32R, name="recr")
                            r_sb = rb.tile([128, 512], F32, name="r")
                            nc.vector.reciprocal(rec_f[:], ps_d[:])
                            with nc.allow_low_precision("f32r recip broadcast"):
                                nc.vector.tensor_copy(rec_r[:], rec_f[:])
                            ps_r = psR.tile([128, 512], F32, name="psr")
                            nc.tensor.matmul(
                                ps_r[:], ones_row[:], rec_r[:], start=True, stop=True
                            )
                            nc.vector.tensor_copy(r_sb[:], ps_r[:])
                            nc.vector.tensor_tensor(
                                yt[:, h, qsl], ps_y[:], r_sb[:], ALU.mult
                            )

                # ---------------- Phase C: output projection ----------------
                with (
                    tc.tile_pool(name="ob", bufs=2) as obp,
                    tc.tile_pool(name="psO", bufs=4, space="PSUM") as psO,
                ):
                    for fm in range(C // 128):
                        ob = obp.tile([128, T], F32, name="ob")
                        for n in range(NQ):
                            nsl = bass.ts(n, 512)
                            ps_o = psO.tile([128, 512], F32, name="pso")
                            for h in range(HQ):
                                nc.tensor.matmul(
                                    ps_o[:],
                                    wp_sb[:, h, fm * 128 : (fm + 1) * 128],
                                    yt[:, h, nsl],
                                    start=(h == 0),
                                    stop=(h == HQ - 1),
                                )
                            nc.vector.tensor_copy(ob[:, nsl], ps_o[:])
                        nc.sync.dma_start(outT[fm * 128 : (fm + 1) * 128, :], ob[:])

    nc.compile()
    return nc


def _get_nc():
    global _NC
    if _NC is None:
        _NC = build_nc()
    return _NC


def _prep_inputs(x, w_attn, w_proj):
    """Build the 8 per-core input maps from the full-problem arrays."""
    perm = np.concatenate([np.arange(0, HD, 2), np.arange(1, HD, 2)])

    f = np.arange(64, dtype=np.float64)
    inv = ROPE_THETA ** (-2.0 * f / HD)
    ang = inv[:, None] * np.arange(T, dtype=np.float64)[None, :]
    trigc = np.cos(ang).astype(np.float32)
    trigs = np.sin(ang).astype(np.float32)

    kk = np.arange(128)[None, :, None]
    qq = np.arange(512)[None, None, :]
    dd = np.arange(4)[:, None, None]
    maskd = ((128 * dd + kk) <= qq).astype(BF)

    w_attn = np.asarray(w_attn)
    w_proj = np.asarray(w_proj)
    x = np.asarray(x)

    in_maps = []
    for core in range(N_CORES):
        b, g = core // TP, core % TP
        xT = np.ascontiguousarray(x[b].T).astype(BF)

        qrows = []
        for h in range(HQ):
            gh = g * HQ + h
            qrows.append(gh * HD + perm)
        for kv in range(HKV):
            gk = g * HKV + kv
            qrows.append(N_HEAD * HD + gk * HD + perm)
        qrows = np.concatenate(qrows)
        wqkT = np.ascontiguousarray(w_attn[qrows].T).astype(BF)

        vrows = np.concatenate(
            [
                (N_HEAD + N_KV_HEAD) * HD + (g * HKV + kv) * HD + np.arange(HD)
                for kv in range(HKV)
            ]
        )
        wvT = np.ascontiguousarray(w_attn[vrows].T).astype(BF)

        cols = np.arange(g * HQ * HD, (g + 1) * HQ * HD)
        wpT = np.ascontiguousarray(w_proj[:, cols].T).astype(BF)

        in_maps.append(
            {
                "xT": xT,
                "wqkT": wqkT,
                "wvT": wvT,
                "wpT": wpT,
                "trigc": trigc,
                "trigs": trigs,
                "maskd": maskd,
            }
        )
    return in_maps


def kernel(x, w_attn, w_proj):
    global LAST_RUN
    nc = _get_nc()
    in_maps = _prep_inputs(x, w_attn, w_proj)
    res = run_bass_kernel_spmd(nc, in_maps, core_ids=list(range(N_CORES)))
    LAST_RUN = res
    out = np.empty((B, T, C), dtype=np.float32)
    for b in range(B):
        acc = res.results[TP * b]["outT"] + res.results[TP * b + 1]["outT"]
        out[b] = acc.T
    return out


# revision 16
# speedup vs baseline: 1.4158x; 1.4158x over previous
"""Causal self-attention (GQA + RoPE) Trainium2 kernel, 8-way sharded.

Sharding: DP=4 over batch x TP=2 over kv-head groups (2 kv heads + their
8 q heads per group).  Each core computes its batch's qkv projection for
its head group, causal flash-style attention, and a partial c_proj
(columns of w_proj for its head group).  Host sums the two partial
c_proj outputs per batch.

Everything on-chip runs transposed ([feature, token] layout) so matmuls
contract along partitions; host transposes inputs/outputs.

RoPE trick: w_attn q/k rows are permuted per-head to [even dims; odd
dims] so the rotation pairs (2f, 2f+1) land in partition f and f+64 of
the qkv psum tile; the rotation is then 6 DVE ops per tile (PSUM inputs
may sit at a different base partition than SBUF inputs, which makes
this legal).  The q.k dot product is invariant to the shared
permutation; v is unpermuted.

Softmax: att^T tiles ([k, q] layout) are exp'd on ACT without
max-subtraction (logits are O(6), fp32-safe); denominators accumulate
via ones-column matmuls; the per-q reciprocal is broadcast down
partitions with a f32r outer-product matmul.
"""

import math

import numpy as np
import ml_dtypes

import concourse.bass as bass
import concourse.mybir as mybir
import concourse.tile as tile
from concourse import bacc
from concourse.bass_utils import run_bass_kernel_spmd

ALU = mybir.AluOpType
AF = mybir.ActivationFunctionType
F32 = mybir.dt.float32
F32R = mybir.dt.float32r
BF16 = mybir.dt.bfloat16
BF = ml_dtypes.bfloat16

# problem shape (hardcoded per contest rules)
B, T, C = 4, 2048, 2048
N_HEAD, N_KV_HEAD, HD = 16, 4, 128
ROPE_THETA = 10000.0

TP = 2            # head-group shards
DP = 4            # batch shards
HQ = N_HEAD // TP         # 8 q heads per core
HKV = N_KV_HEAD // TP     # 2 kv heads per core
NREP = N_HEAD // N_KV_HEAD  # 4
QK_ROWS = (HQ + HKV) * HD   # 1280
KC = C // 128     # 16 contraction tiles
NQ = T // 512     # 4 token strips
MQK = QK_ROWS // 128  # 10 feature tiles (8 q heads + 2 kv heads)
SCALE = 1.0 / math.sqrt(HD)

N_CORES = 8

_NC = None        # cached compiled Bass module
LAST_RUN = None   # BassKernelResults of the most recent kernel() call


def build_nc(dbg=False):
    nc = bacc.Bacc(None, target_bir_lowering=False, debug=False)

    xT = nc.declare_dram_parameter("xT", [C, T], BF16, isOutput=False)
    wqkT = nc.declare_dram_parameter("wqkT", [C, QK_ROWS], BF16, isOutput=False)
    wvT = nc.declare_dram_parameter("wvT", [C, HKV * HD], BF16, isOutput=False)
    wpT = nc.declare_dram_parameter("wpT", [HQ * HD, C], BF16, isOutput=False)
    # trigF = [cos; sin] stacked on partitions, trigW = [sin; cos]
    trigf = nc.declare_dram_parameter("trigf", [128, T], F32, isOutput=False)
    trigw = nc.declare_dram_parameter("trigw", [128, T], F32, isOutput=False)
    maskd = nc.declare_dram_parameter("maskd", [4, 128, 512], BF16, isOutput=False)
    outT = nc.declare_dram_parameter("outT", [C, T], F32, isOutput=True)
    if dbg:
        dbg_q = nc.declare_dram_parameter("dbg_q", [128, T], BF16, isOutput=True)
        dbg_k = nc.declare_dram_parameter("dbg_k", [128, T], BF16, isOutput=True)
        dbg_v = nc.declare_dram_parameter("dbg_v", [128, T // 128, HKV * HD], BF16, isOutput=True)
        dbg_y = nc.declare_dram_parameter("dbg_y", [128, HQ, T], BF16, isOutput=True)

    with tile.TileContext(nc) as tc:
        with (
            tc.tile_pool(name="const", bufs=1) as const,
            tc.tile_pool(name="persist", bufs=1) as persist,
        ):
            trigf_sb = const.tile([128, T], F32, name="trigf")
            trigw_sb = const.tile([128, T], F32, name="trigw")
            mask_sb = const.tile([128, 4, 512], BF16, name="mask")
            ones_col = const.tile([128, 1], BF16, name="onec")
            ones_row_f = const.tile([1, 128], F32, name="onerf")
            ones_row = const.tile([1, 128], F32R, name="oner")

            nc.sync.dma_start(trigf_sb[:], trigf[:])
            nc.sync.dma_start(trigw_sb[:], trigw[:])
            nc.sync.dma_start(mask_sb[:], maskd.rearrange("d p q -> p d q"))
            nc.vector.memset(ones_col[:], 1.0)
            nc.vector.memset(ones_row_f[:], 1.0)
            with nc.allow_low_precision("f32r ones for recip broadcast"):
                nc.vector.tensor_copy(ones_row[:], ones_row_f[:])

            qrot = [persist.tile([128, T], BF16, name=f"qrot{h}") for h in range(HQ)]
            krot = [persist.tile([128, T], BF16, name=f"krot{h}") for h in range(HKV)]
            v_sb = persist.tile([128, T // 128, HKV * HD], BF16, name="vtok")
            yt = persist.tile([128, HQ, T], BF16, name="yt")

            # ---------------- Phase A: qkv projection + RoPE ----------------
            with (
                tc.tile_pool(name="wa", bufs=1) as wa,
                tc.tile_pool(name="xa", bufs=2) as xa,
                tc.tile_pool(name="ta", bufs=2) as ta,
                tc.tile_pool(name="psA", bufs=2, space="PSUM") as psA,
                tc.tile_pool(name="psP2", bufs=2, space="PSUM") as psP2,
                tc.tile_pool(name="psV", bufs=2, space="PSUM") as psV,
            ):
                wqk_sb = wa.tile([128, KC, QK_ROWS], BF16, name="wqk")
                wv_sb = wa.tile([128, KC, HKV * HD], BF16, name="wv")
                for kc in range(KC):
                    nc.sync.dma_start(
                        wqk_sb[:, kc, :], wqkT[kc * 128 : (kc + 1) * 128, :]
                    )
                    nc.sync.dma_start(
                        wv_sb[:, kc, :], wvT[kc * 128 : (kc + 1) * 128, :]
                    )

                for n in range(NQ):
                    nsl = bass.ts(n, 512)
                    xs = xa.tile([128, KC, 512], BF16, name="xs")
                    for kc in range(KC):
                        nc.sync.dma_start(
                            xs[:, kc, :], xT[kc * 128 : (kc + 1) * 128, nsl]
                        )
                    # kv heads first (m order 8,9,0..7) so phase B can start early
                    for m in [HQ, HQ + 1] + list(range(HQ)):
                        ps = psA.tile([128, 512], F32, name="psA")
                        for kc in range(KC):
                            nc.tensor.matmul(
                                ps[:],
                                wqk_sb[:, kc, m * 128 : (m + 1) * 128],
                                xs[:, kc, :],
                                start=(kc == 0),
                                stop=(kc == KC - 1),
                            )
                        dst = qrot[m] if m < HQ else krot[m - HQ]
                        # RoPE: rows [even dims x1; odd dims x2] of the head.
                        #   P  = ps * [c; c] = [x1*c; x2*c]   (SBUF f32)
                        #   P2 = ps * [s; s] = [x1*s; x2*s]   (PSUM f32)
                        #   out[0:64]   = P[0:64] - P2[64:128]  = x1*c - x2*s
                        #   out[64:128] = P2[0:64] + P[64:128]  = x1*s + x2*c
                        # (P/P2 split across SBUF+PSUM so each combine op reads
                        #  one SBUF + one PSUM operand at different bases.)
                        pt = ta.tile([128, 512], F32, name="pt")
                        p2 = psP2.tile([128, 512], F32, name="p2")
                        nc.vector.tensor_tensor(
                            pt[:], ps[:], trigf_sb[:, nsl], ALU.mult
                        )
                        nc.vector.tensor_tensor(
                            p2[:], ps[:], trigw_sb[:, nsl], ALU.mult
                        )
                        nc.vector.tensor_tensor(
                            dst[0:64, nsl], pt[0:64, :], p2[64:128, :], ALU.subtract
                        )
                        nc.vector.tensor_tensor(
                            dst[64:128, nsl], p2[0:64, :], pt[64:128, :], ALU.add
                        )
                    # v for this strip, token-major
                    for j in range(4):
                        tt = n * 4 + j
                        psv = psV.tile([128, HKV * HD], F32, name="psv")
                        for kc in range(KC):
                            nc.tensor.matmul(
                                psv[:],
                                xs[:, kc, j * 128 : (j + 1) * 128],
                                wv_sb[:, kc, :],
                                start=(kc == 0),
                                stop=(kc == KC - 1),
                            )
                        nc.scalar.copy(v_sb[:, tt, :], psv[:])

            # ---------------- Phase B: causal attention ----------------
            with tc.tile_pool(name="wp", bufs=1) as wp:
                # prefetch w_proj during phase B
                wp_sb = wp.tile([128, HQ, C], BF16, name="wp")
                for h in range(HQ):
                    nc.sync.dma_start(
                        wp_sb[:, h, :], wpT[h * 128 : (h + 1) * 128, :]
                    )

                with (
                    tc.tile_pool(name="eb", bufs=4) as eb,
                    tc.tile_pool(name="rb", bufs=2) as rb,
                    tc.tile_pool(name="psS", bufs=3, space="PSUM") as psS,
                    tc.tile_pool(name="psY", bufs=2, space="PSUM") as psY,
                    tc.tile_pool(name="psD", bufs=2, space="PSUM") as psD,
                    tc.tile_pool(name="psR", bufs=1, space="PSUM") as psR,
                ):
                    for h in range(HQ):
                        kvh = h // NREP
                        for qj in range(NQ):
                            qsl = bass.ts(qj, 512)
                            ps_y = psY.tile([128, 512], F32, name="psy")
                            ps_d = psD.tile([1, 512], F32, name="psd")
                            nkt = 4 * qj + 4
                            for kt in range(nkt):
                                ps_s = psS.tile([128, 512], F32, name="pss")
                                nc.tensor.matmul(
                                    ps_s[:],
                                    krot[kvh][:, kt * 128 : (kt + 1) * 128],
                                    qrot[h][:, qsl],
                                    start=True,
                                    stop=True,
                                )
                                e = eb.tile([128, 512], BF16, name="e")
                                nc.scalar.activation(
                                    e[:], ps_s[:], AF.Exp, scale=SCALE
                                )
                                d = kt - 4 * qj
                                if d >= 0:
                                    nc.vector.tensor_tensor(
                                        e[:], e[:], mask_sb[:, d, :], ALU.mult
                                    )
                                nc.tensor.matmul(
                                    ps_y[:],
                                    v_sb[:, kt, kvh * HD : (kvh + 1) * HD],
                                    e[:],
                                    start=(kt == 0),
                                    stop=(kt == nkt - 1),
                                )
                                nc.tensor.matmul(
                                    ps_d[:],
                                    ones_col[:],
                                    e[:],
                                    start=(kt == 0),
                                    stop=(kt == nkt - 1),
                                )
                            rec_f = rb.tile([1, 512], F32, name="recf")
                            rec_r = rb.tile([1, 512], F32R, name="recr")
                            r_sb = rb.tile([128, 512], F32, name="r")
                            nc.vector.reciprocal(rec_f[:], ps_d[:])
                            with nc.allow_low_precision("f32r recip broadcast"):
                                nc.vector.tensor_copy(rec_r[:], rec_f[:])
                            ps_r = psR.tile([128, 512], F32, name="psr")
                            nc.tensor.matmul(
                                ps_r[:], ones_row[:], rec_r[:], start=True, stop=True
                            )
                            nc.scalar.copy(r_sb[:], ps_r[:])
                            nc.vector.tensor_tensor(
                                yt[:, h, qsl], ps_y[:], r_sb[:], ALU.mult
                            )

                if dbg:
                    nc.sync.dma_start(dbg_q[:], qrot[0][:])
                    nc.sync.dma_start(dbg_k[:], krot[0][:])
                    nc.sync.dma_start(dbg_v[:], v_sb[:])
                    nc.sync.dma_start(dbg_y[:], yt[:])

                # ---------------- Phase C: output projection ----------------
                with (
                    tc.tile_pool(name="ob", bufs=2) as obp,
                    tc.tile_pool(name="psO", bufs=4, space="PSUM") as psO,
                ):
                    for fm in range(C // 128):
                        ob = obp.tile([128, T], F32, name="ob")
                        for n in range(NQ):
                            nsl = bass.ts(n, 512)
                            ps_o = psO.tile([128, 512], F32, name="pso")
                            for h in range(HQ):
                                nc.tensor.matmul(
                                    ps_o[:],
                                    wp_sb[:, h, fm * 128 : (fm + 1) * 128],
                                    yt[:, h, nsl],
                                    start=(h == 0),
                                    stop=(h == HQ - 1),
                                )
                            nc.scalar.copy(ob[:, nsl], ps_o[:])
                        nc.sync.dma_start(outT[fm * 128 : (fm + 1) * 128, :], ob[:])

    nc.compile()
    return nc


def _get_nc():
    global _NC
    if _NC is None:
        _NC = build_nc()
    return _NC


def _prep_inputs(x, w_attn, w_proj):
    """Build the 8 per-core input maps from the full-problem arrays."""
    perm = np.concatenate([np.arange(0, HD, 2), np.arange(1, HD, 2)])

    f = np.arange(64, dtype=np.float64)
    inv = ROPE_THETA ** (-2.0 * f / HD)
    ang = inv[:, None] * np.arange(T, dtype=np.float64)[None, :]
    trigc = np.cos(ang).astype(np.float32)
    trigs = np.sin(ang).astype(np.float32)
    trigf = np.concatenate([trigc, trigc], axis=0)  # [128, T] = [c; c]
    trigw = np.concatenate([trigs, trigs], axis=0)  # [128, T] = [s; s]

    kk = np.arange(128)[None, :, None]
    qq = np.arange(512)[None, None, :]
    dd = np.arange(4)[:, None, None]
    maskd = ((128 * dd + kk) <= qq).astype(BF)

    w_attn = np.asarray(w_attn)
    w_proj = np.asarray(w_proj)
    x = np.asarray(x)

    in_maps = []
    for core in range(N_CORES):
        b, g = core // TP, core % TP
        xT = np.ascontiguousarray(x[b].T).astype(BF)

        qrows = []
        for h in range(HQ):
            gh = g * HQ + h
            qrows.append(gh * HD + perm)
        for kv in range(HKV):
            gk = g * HKV + kv
            qrows.append(N_HEAD * HD + gk * HD + perm)
        qrows = np.concatenate(qrows)
        wqkT = np.ascontiguousarray(w_attn[qrows].T).astype(BF)

        vrows = np.concatenate(
            [
                (N_HEAD + N_KV_HEAD) * HD + (g * HKV + kv) * HD + np.arange(HD)
                for kv in range(HKV)
            ]
        )
        wvT = np.ascontiguousarray(w_attn[vrows].T).astype(BF)

        cols = np.arange(g * HQ * HD, (g + 1) * HQ * HD)
        wpT = np.ascontiguousarray(w_proj[:, cols].T).astype(BF)

        in_maps.append(
            {
                "xT": xT,
                "wqkT": wqkT,
                "wvT": wvT,
                "wpT": wpT,
                "trigf": trigf,
                "trigw": trigw,
                "maskd": maskd,
            }
        )
    return in_maps


def kernel(x, w_attn, w_proj):
    global LAST_RUN
    nc = _get_nc()
    in_maps = _prep_inputs(x, w_attn, w_proj)
    res = run_bass_kernel_spmd(nc, in_maps, core_ids=list(range(N_CORES)))
    LAST_RUN = res
    out = np.empty((B, T, C), dtype=np.float32)
    for b in range(B):
        acc = res.results[TP * b]["outT"] + res.results[TP * b + 1]["outT"]
        out[b] = acc.T
    return out


# revision 23
# speedup vs baseline: 1.4655x; 1.0351x over previous
"""Causal self-attention (GQA + RoPE) Trainium2 kernel, 8-way sharded.

Sharding: DP=4 over batch x TP=2 over kv-head groups (2 kv heads + their
8 q heads per group).  Each core computes its batch's qkv projection for
its head group, causal attention, and a partial c_proj (columns of
w_proj for its head group).  Host sums the two partial c_proj outputs
per batch.

Everything on-chip runs transposed ([feature, token] layout) so matmuls
contract along partitions; host transposes inputs/outputs.

Pipeline: the attention inner loop is ACT-bound (one exp per QK tile),
so the q/k projection + RoPE work for head h+1 is interleaved into the
PE stream of head h's attention, keeping the PE busy while ACT churns
through exps.

RoPE: w_attn q/k rows are permuted per-head to [even dims; odd dims] so
rotation pairs land at partition f and f+64 of the qkv psum tile:
  P  = ps * [c; c] (SBUF),  P2 = ps * [s; s] (PSUM)
  out[0:64]   = P[0:64]  - P2[64:128]
  out[64:128] = P2[0:64] + P[64:128]
(each combine reads one SBUF + one PSUM operand, which may sit at
different base partitions; two SBUF operands may not).

Softmax: att^T tiles ([k, q] layout) are exp'd on ACT without
max-subtraction (logits are O(6), fp32-safe).  Denominators: groups of
4 e-tiles are tree-summed on DVE and hit with one ones-column matmul
per group (deferred into the next group's PE stream); the per-q
reciprocal is broadcast down partitions with a f32r outer-product
matmul, also deferred one q-tile.
"""

import math

import numpy as np
import ml_dtypes

import concourse.bass as bass
import concourse.mybir as mybir
import concourse.tile as tile
from concourse import bacc
from concourse.bass_utils import run_bass_kernel_spmd

ALU = mybir.AluOpType
AF = mybir.ActivationFunctionType
F32 = mybir.dt.float32
F32R = mybir.dt.float32r
BF16 = mybir.dt.bfloat16
BF = ml_dtypes.bfloat16

# problem shape (hardcoded per contest rules)
B, T, C = 4, 2048, 2048
N_HEAD, N_KV_HEAD, HD = 16, 4, 128
ROPE_THETA = 10000.0

TP = 2            # head-group shards
DP = 4            # batch shards
HQ = N_HEAD // TP         # 8 q heads per core
HKV = N_KV_HEAD // TP     # 2 kv heads per core
NREP = N_HEAD // N_KV_HEAD  # 4
QK_ROWS = (HQ + HKV) * HD   # 1280
KC = C // 128     # 16 contraction tiles
NQ = T // 512     # 4 token strips
MQK = QK_ROWS // 128  # 10 feature tiles (8 q heads + 2 kv heads)
FM = C // 128     # 16 output feature tiles
SCALE = 1.0 / math.sqrt(HD)

N_CORES = 8

_NC = None        # cached compiled Bass module
LAST_RUN = None   # BassKernelResults of the most recent kernel() call


def build_nc(dbg=False):
    nc = bacc.Bacc(None, target_bir_lowering=False, debug=False)

    xT = nc.declare_dram_parameter("xT", [C, T], BF16, isOutput=False)
    # wqk3[m, p, kc*128+col] = w_qk_perm.T[kc*128+p, m*128+col]
    wqk3 = nc.declare_dram_parameter("wqk3", [MQK, 128, C], BF16, isOutput=False)
    # wv3[p, kc*256+c] = w_v.T[kc*128+p, c]
    wv3 = nc.declare_dram_parameter("wv3", [128, KC * HKV * HD], BF16, isOutput=False)
    # wp5[fm, d, h, p] = w_proj[fm*128+p, g*1024 + h*128 + d]
    wp5 = nc.declare_dram_parameter("wp5", [FM, 128, HQ, 128], BF16, isOutput=False)
    trigf = nc.declare_dram_parameter("trigf", [128, T], F32, isOutput=False)  # [c;c]
    trigw = nc.declare_dram_parameter("trigw", [128, T], F32, isOutput=False)  # [s;s]
    maskd = nc.declare_dram_parameter("maskd", [4, 128, 512], BF16, isOutput=False)
    outT = nc.declare_dram_parameter("outT", [C, T], F32, isOutput=True)
    if dbg:
        dbg_q = nc.declare_dram_parameter("dbg_q", [128, T], BF16, isOutput=True)
        dbg_k = nc.declare_dram_parameter("dbg_k", [128, T], BF16, isOutput=True)
        dbg_v = nc.declare_dram_parameter(
            "dbg_v", [128, T // 128, HKV * HD], BF16, isOutput=True
        )
        dbg_y = nc.declare_dram_parameter("dbg_y", [128, HQ, T], BF16, isOutput=True)

    with tile.TileContext(nc) as tc:
        with (
            tc.tile_pool(name="const", bufs=1) as const,
            tc.tile_pool(name="persist", bufs=1) as persist,
        ):
            trigf_sb = const.tile([128, T], F32, name="trigf")
            trigw_sb = const.tile([128, T], F32, name="trigw")
            mask_sb = const.tile([128, 4, 512], BF16, name="mask")
            ones_col = const.tile([128, 1], BF16, name="onec")
            ones_row_f = const.tile([1, 128], F32, name="onerf")
            ones_row = const.tile([1, 128], F32R, name="oner")

            qrot = [persist.tile([128, T], BF16, name=f"qrot{h}") for h in range(HQ)]
            krot = [persist.tile([128, T], BF16, name=f"krot{h}") for h in range(HKV)]
            v_sb = persist.tile([128, T // 128, HKV * HD], BF16, name="vtok")
            yt = persist.tile([128, HQ, T], BF16, name="yt")

            with (
                tc.tile_pool(name="xa", bufs=1) as xa,
                tc.tile_pool(name="wm", bufs=3) as wm,
                tc.tile_pool(name="ta", bufs=1) as ta,
                tc.tile_pool(name="psA", bufs=1, space="PSUM") as psA,
                tc.tile_pool(name="psP2", bufs=1, space="PSUM") as psP2,
            ):
                xs = xa.tile([128, KC, T], BF16, name="xs")

                def load_wm(m):
                    w = wm.tile([128, KC, 128], BF16, name="wm")
                    wsrc = wqk3[m, :, :].rearrange("p (kc c) -> p kc c", kc=KC)
                    for i in range(4):
                        nc.sync.dma_start(
                            w[:, 4 * i : 4 * i + 4, :], wsrc[:, 4 * i : 4 * i + 4, :]
                        )
                    return w

                def emit_rope(m, n, ps):
                    dst = qrot[m] if m < HQ else krot[m - HQ]
                    nsl = bass.ts(n, 512)
                    pt = ta.tile([128, 512], F32, name="pt")
                    p2 = psP2.tile([128, 512], F32, name="p2")
                    nc.vector.tensor_tensor(pt[:], ps[:], trigf_sb[:, nsl], ALU.mult)
                    nc.vector.tensor_tensor(p2[:], ps[:], trigw_sb[:, nsl], ALU.mult)
                    nc.vector.tensor_tensor(
                        dst[0:64, nsl], pt[0:64, :], p2[64:128, :], ALU.subtract
                    )
                    nc.vector.tensor_tensor(
                        dst[64:128, nsl], p2[0:64, :], pt[64:128, :], ALU.add
                    )

                def a_stream(m, pool):
                    """Generator emitting projection + RoPE for feature tile m,
                    yielding at pacing points for interleaving."""
                    w = load_wm(m)
                    yield
                    for n in range(NQ):
                        nsl = bass.ts(n, 512)
                        ps = pool.tile([128, 512], F32, name="psA")
                        for kc in range(KC):
                            nc.tensor.matmul(
                                ps[:],
                                w[:, kc, :],
                                xs[:, kc, nsl],
                                start=(kc == 0),
                                stop=(kc == KC - 1),
                            )
                            if kc % 2 == 1:
                                yield
                        nsl2 = bass.ts(n, 512)
                        dst = qrot[m] if m < HQ else krot[m - HQ]
                        pt = ta.tile([128, 512], F32, name="pt")
                        p2 = psP2.tile([128, 512], F32, name="p2")
                        nc.vector.tensor_tensor(
                            pt[:], ps[:], trigf_sb[:, nsl2], ALU.mult
                        )
                        yield
                        nc.vector.tensor_tensor(
                            p2[:], ps[:], trigw_sb[:, nsl2], ALU.mult
                        )
                        yield
                        nc.vector.tensor_tensor(
                            dst[0:64, nsl2], pt[0:64, :], p2[64:128, :], ALU.subtract
                        )
                        yield
                        nc.vector.tensor_tensor(
                            dst[64:128, nsl2], p2[0:64, :], pt[64:128, :], ALU.add
                        )
                        yield

                def drain(gen):
                    for _ in gen:
                        pass

                # ---- A0: v projection + k heads + q head 0 (pure PE phase) ----
                with (
                    tc.tile_pool(name="wvp", bufs=1) as wvp,
                    tc.tile_pool(name="psV", bufs=2, space="PSUM") as psV,
                    tc.tile_pool(name="psA0", bufs=2, space="PSUM") as psA0,
                ):
                    wv_sb = wvp.tile([128, KC, HKV * HD], BF16, name="wv")
                    wvsrc = wv3.rearrange("p (kc c) -> p kc c", kc=KC)
                    for i in range(4):
                        nc.sync.dma_start(
                            wv_sb[:, 4 * i : 4 * i + 4, :],
                            wvsrc[:, 4 * i : 4 * i + 4, :],
                        )
                    for kc in range(KC):
                        nc.sync.dma_start(
                            xs[:, kc, bass.ts(0, 512)],
                            xT[kc * 128 : (kc + 1) * 128, bass.ts(0, 512)],
                        )
                    nc.sync.dma_start(trigf_sb[:], trigf[:])
                    nc.sync.dma_start(trigw_sb[:], trigw[:])
                    nc.sync.dma_start(mask_sb[:], maskd.rearrange("d p q -> p d q"))
                    nc.vector.memset(ones_col[:], 1.0)
                    nc.vector.memset(ones_row_f[:], 1.0)
                    with nc.allow_low_precision("f32r ones for recip broadcast"):
                        nc.vector.tensor_copy(ones_row[:], ones_row_f[:])
                    wk0 = load_wm(HQ)
                    wk1 = load_wm(HQ + 1)
                    wq0 = load_wm(0)
                    for n in range(NQ):
                        nsl = bass.ts(n, 512)
                        if n + 1 < NQ:
                            nsl_next = bass.ts(n + 1, 512)
                            for kc in range(KC):
                                nc.sync.dma_start(
                                    xs[:, kc, nsl_next],
                                    xT[kc * 128 : (kc + 1) * 128, nsl_next],
                                )
                        for tt in range(4 * n, 4 * n + 4):
                            psv = psV.tile([128, HKV * HD], F32, name="psv")
                            for kc in range(KC):
                                nc.tensor.matmul(
                                    psv[:],
                                    xs[:, kc, tt * 128 : (tt + 1) * 128],
                                    wv_sb[:, kc, :],
                                    start=(kc == 0),
                                    stop=(kc == KC - 1),
                                )
                            nc.scalar.copy(v_sb[:, tt, :], psv[:])
                        for m, w in ((HQ, wk0), (HQ + 1, wk1), (0, wq0)):
                            ps = psA0.tile([128, 512], F32, name="psA")
                            for kc in range(KC):
                                nc.tensor.matmul(
                                    ps[:],
                                    w[:, kc, :],
                                    xs[:, kc, nsl],
                                    start=(kc == 0),
                                    stop=(kc == KC - 1),
                                )
                            emit_rope(m, n, ps)

                # ---- interleaved attention (head h) + projection (head h+1) ----
                with (
                    tc.tile_pool(name="eb", bufs=6) as eb,
                    tc.tile_pool(name="gag", bufs=2) as gag,
                    tc.tile_pool(name="rb", bufs=2) as rb,
                    tc.tile_pool(name="psS", bufs=2, space="PSUM") as psS,
                    tc.tile_pool(name="psY", bufs=2, space="PSUM") as psY,
                    tc.tile_pool(name="psD", bufs=2, space="PSUM") as psD,
                ):
                    def finalize(h, qj, ps_y, ps_d):
                        rec_f = rb.tile([1, 512], F32, name="recf")
                        rec_r = rb.tile([1, 512], F32R, name="recr")
                        r_sb = rb.tile([128, 512], F32, name="r")
                        nc.vector.reciprocal(rec_f[:], ps_d[:])
                        with nc.allow_low_precision("f32r recip broadcast"):
                            nc.vector.tensor_copy(rec_r[:], rec_f[:])
                        ps_r = psS.tile([128, 512], F32, name="pss")
                        nc.tensor.matmul(
                            ps_r[:], ones_row[:], rec_r[:], start=True, stop=True
                        )
                        nc.scalar.copy(r_sb[:], ps_r[:])
                        nc.vector.tensor_tensor(
                            yt[:, h, bass.ts(qj, 512)], ps_y[:], r_sb[:], ALU.mult
                        )

                    pending = None
                    pending_ones = None  # (gs, start, stop, ps_d)
                    for h in range(HQ):
                        kvh = h // NREP
                        agen = a_stream(h + 1, psA) if h + 1 < HQ else iter(())
                        for qj in range(NQ):
                            qsl = bass.ts(qj, 512)
                            ps_y = psY.tile([128, 512], F32, name="psy")
                            ps_d = psD.tile([1, 512], F32, name="psd")
                            nkt = 4 * qj + 4
                            for kt in range(nkt):
                                ps_s = psS.tile([128, 512], F32, name="pss")
                                nc.tensor.matmul(
                                    ps_s[:],
                                    krot[kvh][:, kt * 128 : (kt + 1) * 128],
                                    qrot[h][:, qsl],
                                    start=True,
                                    stop=True,
                                )
                                e = eb.tile([128, 512], BF16, name="e")
                                nc.scalar.activation(
                                    e[:], ps_s[:], AF.Exp, scale=SCALE
                                )
                                d = kt - 4 * qj
                                if d >= 0:
                                    nc.vector.tensor_tensor(
                                        e[:], e[:], mask_sb[:, d, :], ALU.mult
                                    )
                                nc.tensor.matmul(
                                    ps_y[:],
                                    v_sb[:, kt, kvh * HD : (kvh + 1) * HD],
                                    e[:],
                                    start=(kt == 0),
                                    stop=(kt == nkt - 1),
                                )
                                ph = kt % 4
                                if ph == 0:
                                    g0 = e
                                elif ph == 1:
                                    ga = gag.tile([128, 512], BF16, name="ga")
                                    nc.vector.tensor_tensor(
                                        ga[:], g0[:], e[:], ALU.add
                                    )
                                elif ph == 2:
                                    g2 = e
                                else:
                                    gs = gag.tile([128, 512], BF16, name="gs")
                                    nc.vector.tensor_tensor(
                                        gs[:], g2[:], e[:], ALU.add
                                    )
                                    nc.vector.tensor_tensor(
                                        gs[:], gs[:], ga[:], ALU.add
                                    )
                                    if pending_ones is not None:
                                        po, st, sp, pd = pending_ones
                                        nc.tensor.matmul(
                                            pd[:], ones_col[:], po[:],
                                            start=st, stop=sp,
                                        )
                                    grp = kt // 4
                                    pending_ones = (
                                        gs, grp == 0, grp == nkt // 4 - 1, ps_d
                                    )
                                next(agen, None)  # interleave A(h+1) PE work
                                if kt < 5:
                                    next(agen, None)
                            if pending is not None:
                                finalize(*pending)
                            pending = (h, qj, ps_y, ps_d)
                    po, st, sp, pd = pending_ones
                    nc.tensor.matmul(pd[:], ones_col[:], po[:], start=st, stop=sp)
                    finalize(*pending)

                if dbg:
                    nc.sync.dma_start(dbg_q[:], qrot[0][:])
                    nc.sync.dma_start(dbg_k[:], krot[0][:])
                    nc.sync.dma_start(dbg_v[:], v_sb[:])
                    nc.sync.dma_start(dbg_y[:], yt[:])

            # ---------------- Phase C: output projection ----------------
            with (
                tc.tile_pool(name="wpc", bufs=3) as wpc,
                tc.tile_pool(name="ob", bufs=2) as obp,
                tc.tile_pool(name="psO", bufs=4, space="PSUM") as psO,
            ):
                for fm in range(FM):
                    wmc = wpc.tile([128, HQ, 128], BF16, name="wpc")
                    nc.sync.dma_start(wmc[:], wp5[fm, :, :, :])
                    ob = obp.tile([128, T], F32, name="ob")
                    for n in range(NQ):
                        nsl = bass.ts(n, 512)
                        ps_o = psO.tile([128, 512], F32, name="pso")
                        for h in range(HQ):
                            nc.tensor.matmul(
                                ps_o[:],
                                wmc[:, h, :],
                                yt[:, h, nsl],
                                start=(h == 0),
                                stop=(h == HQ - 1),
                            )
                        nc.scalar.copy(ob[:, nsl], ps_o[:])
                    nc.sync.dma_start(outT[fm * 128 : (fm + 1) * 128, :], ob[:])

    nc.compile()
    return nc


def _get_nc():
    global _NC
    if _NC is None:
        _NC = build_nc()
    return _NC


def _prep_inputs(x, w_attn, w_proj):
    """Build the 8 per-core input maps from the full-problem arrays."""
    perm = np.concatenate([np.arange(0, HD, 2), np.arange(1, HD, 2)])

    f = np.arange(64, dtype=np.float64)
    inv = ROPE_THETA ** (-2.0 * f / HD)
    ang = inv[:, None] * np.arange(T, dtype=np.float64)[None, :]
    trigc = np.cos(ang).astype(np.float32)
    trigs = np.sin(ang).astype(np.float32)
    trigf = np.ascontiguousarray(np.concatenate([trigc, trigc], axis=0))
    trigw = np.ascontiguousarray(np.concatenate([trigs, trigs], axis=0))

    kk = np.arange(128)[None, :, None]
    qq = np.arange(512)[None, None, :]
    dd = np.arange(4)[:, None, None]
    maskd = ((128 * dd + kk) <= qq).astype(BF)

    w_attn = np.asarray(w_attn)
    w_proj = np.asarray(w_proj)
    x = np.asarray(x)

    in_maps = []
    for core in range(N_CORES):
        b, g = core // TP, core % TP
        xTa = np.ascontiguousarray(x[b].T).astype(BF)

        qrows = []
        for h in range(HQ):
            gh = g * HQ + h
            qrows.append(gh * HD + perm)
        for kv in range(HKV):
            gk = g * HKV + kv
            qrows.append(N_HEAD * HD + gk * HD + perm)
        qrows = np.concatenate(qrows)
        wqk = w_attn[qrows].astype(BF)  # [1280, C]
        # wqk3[m, p, kc*128+col] = wqk[m*128+col, kc*128+p]
        wqk3 = np.ascontiguousarray(
            wqk.reshape(MQK, 128, KC, 128).transpose(0, 3, 2, 1).reshape(MQK, 128, C)
        )

        vrows = np.concatenate(
            [
                (N_HEAD + N_KV_HEAD) * HD + (g * HKV + kv) * HD + np.arange(HD)
                for kv in range(HKV)
            ]
        )
        wv = w_attn[vrows].astype(BF)  # [256, C]
        # wv3[p, kc*256+c] = wv[c, kc*128+p]
        wv3 = np.ascontiguousarray(
            wv.reshape(HKV * HD, KC, 128).transpose(2, 1, 0).reshape(128, KC * HKV * HD)
        )

        cols = np.arange(g * HQ * HD, (g + 1) * HQ * HD)
        wpg = w_proj[:, cols].astype(BF)  # [C, 1024], rows = out features
        # wp5[fm, d, h, p] = wpg[fm*128+p, h*128+d]
        wp5 = np.ascontiguousarray(
            wpg.T.reshape(HQ, 128, FM, 128).transpose(2, 1, 0, 3)
        )

        in_maps.append(
            {
                "xT": xTa,
                "wqk3": wqk3,
                "wv3": wv3,
                "wp5": wp5,
                "trigf": trigf,
                "trigw": trigw,
                "maskd": maskd,
            }
        )
    return in_maps


def kernel(x, w_attn, w_proj):
    global LAST_RUN
    nc = _get_nc()
    in_maps = _prep_inputs(x, w_attn, w_proj)
    res = run_bass_kernel_spmd(nc, in_maps, core_ids=list(range(N_CORES)))
    LAST_RUN = res
    out = np.empty((B, T, C), dtype=np.float32)
    for b in range(B):
        acc = res.results[TP * b]["outT"] + res.results[TP * b + 1]["outT"]
        out[b] = acc.T
    return out
